# revision 56
# baseline (speedup 1.0000x reference)
"""Trainium2 Bass kernel for nn_DNN_Model_64793876628183.

8-core data-parallel over the batch (16 samples per core):
  - 5-layer MLP (303->1024->1024->1024->512->264) on TensorE, fp16
    weights/activations with fp32 PSUM accumulation.
  - theta = normalize(theta1[:100] + i*theta1[100:200]).
  - real(theta^H T theta) = <Tr, M1> + <Ti, M2> with rank-2 M1 = a a^T + b b^T,
    M2[m,n] = a_m b_n - b_m a_n (outer products via K=2 matmuls on chip).
  - T is re-laid out host-side to TH[b, m, n, t, c] (t in {re, im}, c = 64
    constraints) and cast to fp16, so the big contraction streams T through
    the PE as the moving operand at 1 col/cycle: per sample, 200
    PSUM-accumulated matmuls (lhsT = M12[:, n, t] single column, rhs =
    slab[:, (2n+t)*64 : +64]) land ccc[b, :] directly in one PSUM row.
  - segmented max over c, scale = sqrt(1e-15 * 1/max) via DVE reciprocal +
    ACT sqrt, PE transposes to [batch_p, feat_f], scaled output assembly,
    one DMA out per core.

Everything is DMA-bound: ~48 MB/core (fp16 T re/im + MLP weights) at
~360 GB/s. TimelineSim cost model: ~140.4 us = 2.3 (NEFF/sequencer
startup) + ~133 (gapless DMA stream, PE/DVE/ACT fully hidden under it)
+ ~5.4 tail (scale chain 1.6 + output-DMA latency 1.3 + exit barrier).
"""

import sys

if "/opt/trn_rl_repo" not in sys.path:
    sys.path.insert(0, "/opt/trn_rl_repo")

import json
from contextlib import ExitStack

import numpy as np

import concourse.bass as bass
import concourse.mybir as mybir
import concourse.tile as tile
from concourse.bass_utils import run_bass_kernel_spmd
from concourse.masks import make_identity
from concourse.vector_clock import ScopedClock

# ---------------------------------------------------------------- constants
B = 128
C = 64
NR = 100          # RIS elements
NT_NR4 = 64       # tail width (cols 200:264 of theta1)
IN_DIM = 303
OUT_DIM = 264
N_CORES = 8
B_LOC = B // N_CORES          # 16 samples per core
THRESH_INV = 1.0e15           # 1 / THRESHOLD_W

LAYERS = [(IN_DIM, 1024), (1024, 1024), (1024, 1024), (1024, 512), (512, OUT_DIM)]

T_DT = mybir.dt.float16       # dtype for the streamed T and the M12 weights
T_NP = np.float16
MLP_DT = mybir.dt.float16     # dtype for MLP weights/activations (PSUM fp32)
MLP_NP = np.float16

NQ = NR // 2                  # 50 matmuls per sample (2 n's each)
SLAB_F = NR * 2 * C           # 12800 free columns per sample slab

F32 = mybir.dt.float32

# ------------------------------------------------------------- walrus patch
# The walrus bundled here accepts at most ONE sem-wait per instruction, while
# current Tile attaches several (tail drain, multi-producer instructions).
# Split the excess onto same-engine NoOps that execute immediately before
# (in-order engines => semantically identical).
_MAXW = 1
_split_n = [0]


def _split_excess_waits_json(raw: bytes) -> bytes:
    m = json.loads(raw)
    changed = False
    for fn in m.get("functions", []):
        for blk in fn.get("blocks", []):
            out = []
            for inst in blk.get("instructions", []):
                si = inst.get("sync_info")
                waits = si.get("on_wait") if si else None
                if waits and len(waits) > _MAXW:
                    changed = True
                    extra, keep = waits[:-_MAXW], waits[-_MAXW:]
                    for i in range(0, len(extra), _MAXW):
                        _split_n[0] += 1
                        out.append({
                            "name": f"I-wsplit-{_split_n[0]}",
                            "opcode": "NoOp",
                            "engine": inst["engine"],
                            "ins": [],
                            "outs": [],
                            "debug": inst.get("debug"),
                            "sync_info": {"on_update": [],
                                          "on_wait": extra[i:i + _MAXW]},
                        })
                    si["on_wait"] = keep
                out.append(inst)
            blk["instructions"] = out
    return json.dumps(m).encode() if changed else raw


def _patched_drain_and_barrier(self, tick_clock, wait_clock):
    nc = self.nc
    drain_inst = nc.sync.drain()
    wait_clock.add_sem_waits(drain_inst.ins,
                             ScopedClock({None: tick_clock.global_clock}))
    si = drain_inst.ins.sync_info
    if si is not None and len(si.on_wait) > _MAXW:
        waits = list(si.on_wait)
        drain_inst.ins.sync_info = mybir.SyncInfo(
            on_wait=list(waits[:_MAXW]), on_update=list(si.on_update))
        for i in range(_MAXW, len(waits), _MAXW):
            nop = nc.sync.nop(nofuse=True)
            nop.ins.sync_info = mybir.SyncInfo(
                on_wait=list(waits[i:i + _MAXW]), on_update=[])
    nc.all_engine_barrier()
    assert self.sems is not None
    popped = nc._tile_sem_poison_stack.pop()
    assert popped is self._sem_poison
    nc.clear_and_free_semaphores(list(self.sems.allocated().values()))
    nc.all_engine_barrier()


def _install_patches():
    tile.TileContext._drain_and_barrier = _patched_drain_and_barrier
    orig = bass.Bass.to_json_bytes
    if getattr(bass.Bass, "_wsplit_patched", False):
        return
    def patched(self):
        return _split_excess_waits_json(orig(self))
    bass.Bass.to_json_bytes = patched
    bass.Bass._wsplit_patched = True


# --------------------------------------------------------------- the kernel
def _build_nc() -> bass.Bass:
    _install_patches()
    nc = bass.Bass()

    xT = nc.dram_tensor("xT", [384, B_LOC], MLP_DT, kind="ExternalInput")
    th = nc.dram_tensor("th", [B_LOC, NR, SLAB_F], T_DT, kind="ExternalInput")
    ws = []
    bs = []
    for li, (kdim, mdim) in enumerate(LAYERS):
        ws.append(nc.dram_tensor(f"w{li}", [kdim, mdim], MLP_DT, kind="ExternalInput"))
        bs.append(nc.dram_tensor(f"b{li}", [mdim], F32, kind="ExternalInput"))
    out = nc.dram_tensor("out", [B_LOC, OUT_DIM], F32, kind="ExternalOutput")

    with tile.TileContext(nc) as tc:
        with ExitStack() as ctx:
            _emit(ctx, nc, tc, xT, th, ws, bs, out)
    return nc


def _kchunks(k):
    return [(o, min(128, k - o)) for o in range(0, k, 128)]


def _emit(ctx, nc, tc, xT, th, ws, bs, out):
    consts = ctx.enter_context(tc.tile_pool(name="consts", bufs=1))
    wpool = ctx.enter_context(tc.tile_pool(name="wpool", bufs=3))
    hpool = ctx.enter_context(tc.tile_pool(name="hpool", bufs=1))
    thpool = ctx.enter_context(tc.tile_pool(name="theta", bufs=1))
    m12pool = ctx.enter_context(tc.tile_pool(name="m12", bufs=4))
    upool = ctx.enter_context(tc.tile_pool(name="upool", bufs=4))
    slabs = ctx.enter_context(tc.tile_pool(name="slabs", bufs=4))
    opool = ctx.enter_context(tc.tile_pool(name="opool", bufs=1))

    # PSUM budget (8 banks, pools reserve bufs x max-size per tag statically):
    #   psA (mlp outs + main-loop accumulators, shared tag) 2 banks
    #   ps_tr (transposes) 2, ps_m (M1|M2 combined)         2
    #   ccc_lo + ccc_hi                                     2
    ps_a = ctx.enter_context(tc.tile_pool(name="ps_a", bufs=2, space="PSUM"))
    ps_tr = ctx.enter_context(tc.tile_pool(name="ps_tr", bufs=2, space="PSUM"))
    ps_m = ctx.enter_context(tc.tile_pool(name="ps_m", bufs=2, space="PSUM"))
    ps_ccc = ctx.enter_context(tc.tile_pool(name="ps_ccc", bufs=1, space="PSUM"))

    # ---- constants
    ident = consts.tile([NR, NR], F32, tag="ident")
    make_identity(nc, ident)

    # ---- MLP inputs / weights first (head of the dependency chain), each
    # layer's weights as one batched 3D-AP DMA
    xt = hpool.tile([128, 3, B_LOC], MLP_DT, tag="xt")
    nc.gpsimd.dma_start(out=xt, in_=xT.rearrange("(a p) j -> p a j", p=128))
    x_tiles = [xt[:, 0, :], xt[:, 1, :], xt[:IN_DIM - 256, 2, :]]

    w_tiles = []   # per layer: list of (ko, kc, AP [kc, mdim])
    b_tiles = []   # per layer: list of (mo, mc, AP [mc, 1])
    for li, (kdim, mdim) in enumerate(LAYERS):
        kch = _kchunks(kdim)
        nfull = kdim // 128
        wt = wpool.tile([128, len(kch), mdim], MLP_DT, tag="wt")
        if nfull:
            nc.sync.dma_start(
                out=wt[:, :nfull, :],
                in_=ws[li][:nfull * 128, :].rearrange("(a p) m -> p a m", p=128))
        if kdim % 128:
            nc.sync.dma_start(out=wt[:kdim % 128, nfull, :],
                              in_=ws[li][nfull * 128:, :])
        w_tiles.append([(ko, kc, wt[:kc, i, :]) for i, (ko, kc) in enumerate(kch)])

        if li < 4:
            bt = consts.tile([128, mdim // 128], F32, tag=f"bt{li}")
            nc.sync.dma_start(
                out=bt, in_=bs[li].rearrange("(a p) -> p a", p=128))
            b_tiles.append([(o, c, bt[:, o // 128:o // 128 + 1])
                            for o, c in _kchunks(mdim)])
        else:
            bl = []
            for mo, mc in [(0, NR), (NR, NR), (2 * NR, NT_NR4)]:
                t = consts.tile([mc, 1], F32, tag=f"b{li}_{mo}")
                nc.sync.dma_start(
                    out=t,
                    in_=bs[li][mo:mo + mc].rearrange("(p one) -> p one", one=1))
                bl.append((mo, mc, t))
            b_tiles.append(bl)

    # ---- T slabs stream behind the weights, in pieces so compute starts
    # before the full slab lands (last slabs in quarters: they bound the tail)
    slab_tiles = []
    for b in range(B_LOC):
        slab = slabs.tile([NR, SLAB_F], T_DT, tag="slab")
        npieces = 8
        step = SLAB_F // npieces
        for i in range(npieces):
            nc.sync.dma_start(out=slab[:, i * step:(i + 1) * step],
                              in_=th[b, :, i * step:(i + 1) * step])
        slab_tiles.append(slab)

    # ---- MLP forward: activations stay [feature_p, batch_f]
    h = x_tiles
    for li in range(5):
        kdim, mdim = LAYERS[li]
        new_h = []
        for mo, mc, btile in b_tiles[li]:
            ps = ps_a.tile([mc, B_LOC], F32, tag="ps_a")
            nk = len(w_tiles[li])
            for ki, (ko, kc, wt) in enumerate(w_tiles[li]):
                nc.tensor.matmul(ps, wt[:, mo:mo + mc], h[ki],
                                 start=(ki == 0), stop=(ki == nk - 1))
            hdt = MLP_DT if li < 4 else F32
            ht = hpool.tile([mc, B_LOC], hdt, tag=f"h{li}_{mo}")
            func = (mybir.ActivationFunctionType.Relu if li < 4
                    else mybir.ActivationFunctionType.Identity)
            nc.scalar.activation(out=ht, in_=ps, func=func, bias=btile)
            new_h.append(ht)
        h = new_h

    t_re, t_im, tail = h   # [100,16], [100,16], [64,16] fp32 (theta1 pieces)

    # ---- unit-modulus theta: a + i b
    sq = thpool.tile([NR, B_LOC], F32, tag="sq")
    nc.vector.tensor_mul(sq, t_re, t_re)
    sq2 = thpool.tile([NR, B_LOC], F32, tag="sq2")
    nc.vector.tensor_mul(sq2, t_im, t_im)
    mag2 = thpool.tile([NR, B_LOC], F32, tag="mag2")
    nc.vector.tensor_add(mag2, sq, sq2)
    mag = thpool.tile([NR, B_LOC], F32, tag="mag")
    nc.scalar.activation(out=mag, in_=mag2,
                         func=mybir.ActivationFunctionType.Sqrt)
    rmag = thpool.tile([NR, B_LOC], F32, tag="rmag")
    nc.vector.reciprocal(out=rmag, in_=mag)
    a_t = thpool.tile([NR, B_LOC], F32, tag="a_t")
    nc.vector.tensor_mul(a_t, t_re, rmag)
    b_t = thpool.tile([NR, B_LOC], F32, tag="b_t")
    nc.vector.tensor_mul(b_t, t_im, rmag)

    # interleaved [a|b], [b|a], [a|-b] pairs for per-sample transposes
    nb_t = thpool.tile([NR, B_LOC], F32, tag="nb_t")
    nc.vector.tensor_scalar_mul(nb_t, b_t, -1.0)
    ab_il = thpool.tile([NR, B_LOC, 2], F32, tag="ab_il")
    nc.vector.tensor_copy(ab_il[:, :, 0], a_t)
    nc.vector.tensor_copy(ab_il[:, :, 1], b_t)
    ba_il = thpool.tile([NR, B_LOC, 2], F32, tag="ba_il")
    nc.vector.tensor_copy(ba_il[:, :, 0], b_t)
    nc.vector.tensor_copy(ba_il[:, :, 1], a_t)
    amb_il = thpool.tile([NR, B_LOC, 2], F32, tag="amb_il")
    nc.vector.tensor_copy(amb_il[:, :, 0], a_t)
    nc.vector.tensor_copy(amb_il[:, :, 1], nb_t)

    # ---- output-side transposes (theta^T, tail^T) -> SBUF
    aT_ps = ps_tr.tile([B_LOC, NR], F32, tag="ps_tr")
    nc.tensor.transpose(aT_ps, a_t, ident)
    aT = thpool.tile([B_LOC, NR], F32, tag="aT")
    nc.vector.tensor_copy(aT, aT_ps)
    bT_ps = ps_tr.tile([B_LOC, NR], F32, tag="ps_tr")
    nc.tensor.transpose(bT_ps, b_t, ident)
    bT = thpool.tile([B_LOC, NR], F32, tag="bT")
    nc.vector.tensor_copy(bT, bT_ps)
    tlT_ps = ps_tr.tile([B_LOC, NT_NR4], F32, tag="ps_tr")
    nc.tensor.transpose(tlT_ps, tail, ident[:NT_NR4, :NT_NR4])
    tailT = thpool.tile([B_LOC, NT_NR4], F32, tag="tailT")
    nc.vector.tensor_copy(tailT, tlT_ps)

    # output tile: the pass-through tail columns are final now
    o = opool.tile([B_LOC, OUT_DIM], F32, tag="o")
    nc.vector.tensor_copy(o[:, 2 * NR:], tailT)

    # ---- per-sample quadratic forms
    # two tiles (one PSUM bank holds 512 fp32 columns = 8 samples x 64)
    ccc_lo = ps_ccc.tile([1, 8 * C], F32, tag="ccc_lo")
    ccc_hi = ps_ccc.tile([1, 8 * C], F32, tag="ccc_hi")
    mx = thpool.tile([1, B_LOC], F32, tag="mx")
    for b in range(B_LOC):
        # u_ab = [a; b], u_ba = [b; a], m2l = [a; -b]  as [2, 100]
        uab_ps = ps_tr.tile([2, NR], F32, tag="ps_tr")
        nc.tensor.transpose(uab_ps, ab_il[:, b, :], ident)
        u_ab = upool.tile([2, NR], F32, tag="u_ab")
        nc.vector.tensor_copy(u_ab, uab_ps)
        uba_ps = ps_tr.tile([2, NR], F32, tag="ps_tr")
        nc.tensor.transpose(uba_ps, ba_il[:, b, :], ident)
        u_ba = upool.tile([2, NR], F32, tag="u_ba")
        nc.vector.tensor_copy(u_ba, uba_ps)
        m2l_ps = ps_tr.tile([2, NR], F32, tag="ps_tr")
        nc.tensor.transpose(m2l_ps, amb_il[:, b, :], ident)
        m2l = upool.tile([2, NR], F32, tag="m2l")
        nc.vector.tensor_copy(m2l, m2l_ps)

        # M1 = a a^T + b b^T ; M2[m, n] = a_m b_n - b_m a_n   (both [m_p, n_f])
        m_ps = ps_m.tile([NR, 2 * NR], F32, tag="ps_m")
        nc.tensor.matmul(m_ps[:, 0:NR], u_ab, u_ab, start=True, stop=True)
        nc.tensor.matmul(m_ps[:, NR:2 * NR], m2l, u_ba, start=True, stop=True)
        m12 = m12pool.tile([NR, NR, 2], T_DT, tag="m12")
        nc.vector.tensor_copy(m12[:, :, 0], m_ps[:, 0:NR])
        nc.vector.tensor_copy(m12[:, :, 1], m_ps[:, NR:2 * NR])

        # stream T: 200 matmuls (N=64, M=1) accumulating straight into ccc
        ccc = ccc_lo if b < 8 else ccc_hi
        dst = ccc[:, (b % 8) * C:(b % 8 + 1) * C]
        slab = slab_tiles[b]
        for n in range(NR):
            for t in range(2):
                nc.tensor.matmul(dst, m12[:, n:n + 1, t],
                                 slab[:, (2 * n + t) * C:(2 * n + t + 1) * C],
                                 start=(n == 0 and t == 0),
                                 stop=(n == NR - 1 and t == 1))
        # fold this sample's max immediately: only b15's reduce sits on the
        # critical tail, the rest hide in PE's DMA-wait bubbles
        nc.vector.reduce_max(mx[:, b:b + 1], dst.rearrange("p (o c) -> p o c", o=1),
                             axis=mybir.AxisListType.X)

    # ---- scale = rsqrt(max * 1e15)
    mxT_ps = ps_tr.tile([B_LOC, 1], F32, tag="ps_tr")
    nc.tensor.transpose(mxT_ps, mx, ident[:1, :1])
    # scale = sqrt(1/(max * 1e15)) = sqrt(1e-15 * (1/max))
    rmx = thpool.tile([B_LOC, 1], F32, tag="rmx")
    nc.vector.reciprocal(out=rmx, in_=mxT_ps)
    scale = thpool.tile([B_LOC, 1], F32, tag="scale")
    nc.scalar.activation(out=scale, in_=rmx,
                         func=mybir.ActivationFunctionType.Sqrt,
                         scale=float(1.0 / THRESH_INV))

    # ---- assemble output [16, 264]
    nc.vector.tensor_scalar_mul(o[:, 0:NR], aT, scale)
    nc.vector.tensor_scalar_mul(o[:, NR:2 * NR], bT, scale)
    nc.sync.dma_start(out=out[:, :], in_=o)


_NC_CACHE = None


def _get_nc():
    global _NC_CACHE
    if _NC_CACHE is None:
        _NC_CACHE = _build_nc()
    return _NC_CACHE


def _prep_th(T_real, T_imag, b0, b1):
    # TH[b, m, n, t, c] = T{t}[b, c, n, m], cast to T_NP, flattened to
    # [B_LOC, NR, SLAB_F] with col index n*128 + t*64 + c.
    tr = T_real[b0:b1].astype(T_NP)          # [16, C, n, m]
    ti = T_imag[b0:b1].astype(T_NP)
    th = np.empty((b1 - b0, NR, NR, 2, C), dtype=T_NP)
    th[:, :, :, 0, :] = tr.transpose(0, 3, 2, 1)
    th[:, :, :, 1, :] = ti.transpose(0, 3, 2, 1)
    return np.ascontiguousarray(th.reshape(b1 - b0, NR, SLAB_F))


def make_in_maps(sample1, T_real, T_imag, Ws, bs):
    wmap = {}
    for li, (w, bb) in enumerate(zip(Ws, bs)):
        wmap[f"w{li}"] = np.ascontiguousarray(w, dtype=MLP_NP)
        wmap[f"b{li}"] = np.ascontiguousarray(bb, dtype=np.float32)
    in_maps = []
    for core in range(N_CORES):
        lo, hi = core * B_LOC, (core + 1) * B_LOC
        m = dict(wmap)
        xt = np.zeros((384, B_LOC), dtype=MLP_NP)
        xt[:IN_DIM] = sample1[lo:hi].T.astype(MLP_NP)
        m["xT"] = xt
        m["th"] = _prep_th(T_real, T_imag, lo, hi)
        in_maps.append(m)
    return in_maps


def kernel(sample1, sample2, T_real, T_imag,
           W1, b1, W2, b2, W3, b3, W4, b4, W5, b5):
    nc = _get_nc()
    in_maps = make_in_maps(sample1, T_real, T_imag,
                           [W1, W2, W3, W4, W5], [b1, b2, b3, b4, b5])

    import os
    trace = bool(int(os.environ.get("KERNEL_TRACE", "0")))
    res = run_bass_kernel_spmd(nc, in_maps, core_ids=list(range(N_CORES)),
                               trace=trace)
    global LAST_RESULT
    LAST_RESULT = res
    return np.concatenate([r["out"] for r in res.results], axis=0)


LAST_RESULT = None
